# revision 1
# baseline (speedup 1.0000x reference)
"""Trainium2 Bass kernel for CustomAttention (non-local block).

Reference math (per batch b):
    xf = x.reshape(C, N)                      # C=512, N=H*W=4096
    qT = Wq @ xf + bq                         # [64, N]   (q transposed: d on partitions)
    kT = Wk @ xf + bk                         # [64, N]
    sT[j, i] = sum_d kT[d, j] * qT[d, i]      # scores, keys on partitions
    attn = softmax_j                          # exp / Z (no max-sub needed: |s| < ~15)
    vT[n, e] = sum_c xf[c, n] Wv[e, c] + bv   # v transposed: n on partitions
    out[e, i] = gamma * (sum_j vT[j, e] exp_sT[j, i]) / Z[i] + x[e, i]

Sharding: pure data-parallel — batch b -> NeuronCore b (B == 8 == n_cores).
All matmuls run as float32r (full-rate fp32 on the PE array).
"""

import numpy as np

import concourse.mybir as mybir
import concourse.tile as tile
from concourse import bacc
from concourse.bass_utils import run_bass_kernel_spmd
from concourse.masks import make_identity

B, C, HW, N, D = 8, 512, 64, 4096, 64
P = 128          # partitions
CB = C // P      # 4 channel chunks
JB = N // P      # 32 key chunks
IB = N // 512    # 8 query blocks
NB = 512         # query block width
F32 = mybir.dt.float32
F32R = mybir.dt.float32r
BF16 = mybir.dt.bfloat16

# exposed for test harness
LAST_RESULTS = None


def build_nc(gamma: float):
    nc = bacc.Bacc(None, target_bir_lowering=False)

    x = nc.dram_tensor("x", [C, N], F32, kind="ExternalInput")
    wq = nc.dram_tensor("Wq", [D, C], F32, kind="ExternalInput")
    wk = nc.dram_tensor("Wk", [D, C], F32, kind="ExternalInput")
    wv = nc.dram_tensor("Wv", [C, C], F32, kind="ExternalInput")
    bq = nc.dram_tensor("bq", [D, 1], F32, kind="ExternalInput")
    bk = nc.dram_tensor("bk", [D, 1], F32, kind="ExternalInput")
    bv = nc.dram_tensor("bv", [1, C], F32, kind="ExternalInput")
    out = nc.dram_tensor("out", [C, N], F32, kind="ExternalOutput")

    # x rows grouped as (c p): chunk c holds rows c*128 .. c*128+127
    x_pcn = x[:, :].rearrange("(c p) n -> p c n", p=P)

    from contextlib import ExitStack

    with tile.TileContext(nc) as tc, ExitStack() as stack:
        const = stack.enter_context(tc.tile_pool(name="const", bufs=1))
        qk_pool = stack.enter_context(tc.tile_pool(name="qk", bufs=1))
        vt_pool = stack.enter_context(tc.tile_pool(name="vt", bufs=1))

        wqkT = const.tile([P, CB, P], F32R, tag="wqkT")  # cols 0-63 Wq^T, 64-127 Wk^T
        wvT = const.tile([P, CB, C], F32R, tag="wvT")
        bqk_sb = const.tile([P, 1], F32, tag="bqk")  # rows 0-63 bq, 64-127 bk
        bvb = const.tile([P, C], F32, tag="bvb")
        ones_col = const.tile([P, 32], F32R, tag="ones")
        gamma_col = const.tile([1, P], F32R, tag="gam")

        qT = qk_pool.tile([P, N], F32R, tag="qT")
        kT = qk_pool.tile([P, N], F32R, tag="kT")
        vT = vt_pool.tile([P, JB, C], F32R, tag="vT")

        ones_f32 = const.tile([P, 32], F32, tag="ones_f32")
        nc.vector.memset(ones_f32, 1.0)
        nc.vector.tensor_copy(ones_col, ones_f32)
        gam_f32 = const.tile([1, P], F32, tag="gam_f32")
        nc.vector.memset(gam_f32, gamma)
        nc.vector.tensor_copy(gamma_col, gam_f32)

        # ---------------- phase 0: weights + q/k/v projections ----------------
        with (
            tc.tile_pool(name="ph0", bufs=1) as ph0,
            tc.tile_pool(name="ph0x", bufs=3) as ph0x,
            tc.tile_pool(name="ph0ps", bufs=1, space="PSUM") as ph0ps,
        ):
            ident = ph0.tile([P, P], F32, tag="ident")
            make_identity(nc, ident)

            wq_raw = ph0.tile([D, C], F32, tag="wq_raw")
            wk_raw = ph0.tile([D, C], F32, tag="wk_raw")
            wv_raw = ph0.tile([P, CB, C], F32, tag="wv_raw")  # [e-part, e-chunk, c]
            nc.gpsimd.dma_start(out=wv_raw, in_=wv[:, :].rearrange("(e p) c -> p e c", p=P))
            nc.sync.dma_start(out=wq_raw, in_=wq[:, :])
            nc.sync.dma_start(out=wk_raw, in_=wk[:, :])
            nc.gpsimd.dma_start(out=bqk_sb[0:D, :], in_=bq[:, :])
            nc.gpsimd.dma_start(out=bqk_sb[D:2 * D, :], in_=bk[:, :])
            nc.gpsimd.dma_start(out=bvb, in_=bv[:, :].to_broadcast((P, C)))

            # transpose Wq/Wk: [64, 128c] -> [128c, 64]; q lands in cols 0-63,
            # k in cols 64-127 of the packed weight
            for c in range(CB):
                pq = ph0ps.tile([P, D], F32, tag="wt", bufs=2)
                nc.tensor.transpose(pq, wq_raw[:, c * P:(c + 1) * P], ident[0:D, 0:D])
                nc.vector.tensor_copy(wqkT[:, c, 0:D], pq)
                pk = ph0ps.tile([P, D], F32, tag="wt", bufs=2)
                nc.tensor.transpose(pk, wk_raw[:, c * P:(c + 1) * P], ident[0:D, 0:D])
                nc.vector.tensor_copy(wqkT[:, c, D:2 * D], pk)
            # transpose Wv blocks: in [e-part, c-cols] -> out [c-part, e-cols]
            for c in range(CB):
                for e in range(CB):
                    pv = ph0ps.tile([P, P], F32, tag="wt", bufs=2)
                    nc.tensor.transpose(pv, wv_raw[:, e, c * P:(c + 1) * P], ident)
                    nc.vector.tensor_copy(wvT[:, c, e * P:(e + 1) * P], pv)

            # projections, one 512-wide n-block at a time
            for nb in range(IB):
                ns = slice(nb * NB, (nb + 1) * NB)
                xt = ph0x.tile([P, CB, NB], F32, tag="xt")
                nc.sync.dma_start(out=xt, in_=x_pcn[:, :, ns])
                xtr = ph0x.tile([P, CB, NB], F32R, tag="xtr")
                nc.vector.tensor_copy(xtr, xt)

                psqk = ph0ps.tile([P, NB], F32, tag="q")
                for c in range(CB):
                    nc.tensor.matmul(psqk, wqkT[:, c, :], xtr[:, c, :],
                                     start=(c == 0), stop=(c == CB - 1))
                # engines are lane-locked: q rows live at psum 0-63, k rows at
                # 64-127, so k is written to the HIGH half of kT
                nc.scalar.activation(qT[0:D, ns], psqk[0:D, :],
                                     mybir.ActivationFunctionType.Identity,
                                     bias=bqk_sb[0:D, :])
                nc.scalar.activation(kT[D:2 * D, ns], psqk[D:2 * D, :],
                                     mybir.ActivationFunctionType.Identity,
                                     bias=bqk_sb[D:2 * D, :])
                # incremental row-duplication so copies overlap the matmuls
                nc.sync.dma_start(out=qT[D:2 * D, ns], in_=qT[0:D, ns])
                nc.sync.dma_start(out=kT[0:D, ns], in_=kT[D:2 * D, ns])
                for sub in range(4):
                    jt = nb * 4 + sub
                    sl = slice(sub * P, (sub + 1) * P)
                    psv = ph0ps.tile([P, C], F32, tag="v", bufs=4)
                    for c in range(CB):
                        nc.tensor.matmul(psv, xtr[:, c, sl], wvT[:, c, :],
                                         start=(c == 0), stop=(c == CB - 1))
                    nc.vector.tensor_tensor(vT[:, jt, :], psv, bvb,
                                            op=mybir.AluOpType.add)


        # ---------------- main loop: attention ----------------
        with (
            tc.tile_pool(name="expp", bufs=10) as expp,
            tc.tile_pool(name="xst", bufs=6) as xst,
            tc.tile_pool(name="ost", bufs=6) as ost,
            tc.tile_pool(name="small", bufs=2) as small,
            tc.tile_pool(name="mps", bufs=1, space="PSUM") as mps,
        ):
            for ib in range(IB):
                isl = slice(ib * NB, (ib + 1) * NB)
                psum_z = mps.tile([32, NB], F32, tag="z")
                psum_pv = [mps.tile([P, NB], F32, tag=f"pv{e}", name=f"psum_pv{e}")
                           for e in range(CB)]
                exp_tiles = {}

                def consume_z(jj, psum_z=psum_z, exp_tiles=exp_tiles):
                    # M=32 ones weight: all 32 output rows hold the same
                    # column sum (M=1 drains at half rate on the psum port)
                    nc.tensor.matmul(psum_z, ones_col, exp_tiles[jj],
                                     start=(jj == 0), stop=(jj == JB - 1))

                def consume_pv(jj, psum_pv=psum_pv, exp_tiles=exp_tiles):
                    et = exp_tiles.pop(jj)
                    for e in range(CB):
                        nc.tensor.matmul(psum_pv[e],
                                         vT[:, jj, e * P:(e + 1) * P], et,
                                         start=(jj == 0), stop=(jj == JB - 1))

                for j2 in range(JB // 2):
                    # row-packed pair of K=64 score matmuls (array rows 0-63 /
                    # 64-127) running concurrently on separate psum banks
                    ja, jb = 2 * j2, 2 * j2 + 1
                    psa = mps.tile([P, NB], F32, tag="s", bufs=3, name="psa")
                    psb = mps.tile([P, NB], F32, tag="s", bufs=3, name="psb")
                    nc.tensor.matmul(psa, kT[0:D, ja * P:(ja + 1) * P],
                                     qT[0:D, isl], start=True, stop=True)
                    nc.tensor.matmul(psb, kT[D:2 * D, jb * P:(jb + 1) * P],
                                     qT[D:2 * D, isl], start=True, stop=True,
                                     tile_position=(D, 0))
                    nc.scalar.activation(exp_tiles.setdefault(
                        ja, expp.tile([P, NB], F32R, tag="exp", name="eta")),
                        psa, mybir.ActivationFunctionType.Exp)
                    nc.scalar.activation(exp_tiles.setdefault(
                        jb, expp.tile([P, NB], F32R, tag="exp", name="etb")),
                        psb, mybir.ActivationFunctionType.Exp)
                    if j2 >= 1:
                        consume_z(2 * (j2 - 1))
                        consume_z(2 * (j2 - 1) + 1)
                    if j2 >= 2:
                        consume_pv(2 * (j2 - 2))
                        consume_pv(2 * (j2 - 2) + 1)
                for jj in range(JB - 2, JB):
                    consume_z(jj)
                for jj in range(JB - 4, JB):
                    consume_pv(jj)

                # reciprocal of Z, fold gamma, broadcast across partitions
                # via a K=1 matmul
                rz = small.tile([1, NB], F32R, tag="rz")
                with nc.allow_low_precision(reason="f32r is 32-bit; feeds f32r bcast matmul"):
                    nc.vector.reciprocal(rz, psum_z[0:1, :])
                psrz = mps.tile([P, NB], F32, tag="z")
                nc.tensor.matmul(psrz, gamma_col, rz, start=True, stop=True)
                rzb = small.tile([P, NB], F32, tag="rzb")
                nc.vector.tensor_copy(rzb, psrz)

                for e in range(CB):
                    esl = slice(e * P, (e + 1) * P)
                    xt = xst.tile([P, NB], F32, tag="x")
                    nc.gpsimd.dma_start(out=xt, in_=x[esl, isl])
                    ot = ost.tile([P, NB], F32, tag="o")
                    nc.vector.tensor_tensor(ot, psum_pv[e], rzb,
                                            op=mybir.AluOpType.mult)
                    nc.vector.tensor_tensor(ot, ot, xt, op=mybir.AluOpType.add)
                    nc.sync.dma_start(out=out[esl, isl], in_=ot)

    nc.compile()
    return nc


def kernel(**inputs):
    global LAST_RESULTS
    x = np.asarray(inputs["x"], dtype=np.float32)
    gamma = float(np.asarray(inputs["gamma"]).reshape(-1)[0])

    nc = build_nc(gamma)

    in_maps = []
    for b in range(B):
        in_maps.append({
            "x": np.ascontiguousarray(x[b].reshape(C, N)),
            "Wq": np.ascontiguousarray(inputs["Wq"], dtype=np.float32),
            "Wk": np.ascontiguousarray(inputs["Wk"], dtype=np.float32),
            "Wv": np.ascontiguousarray(inputs["Wv"], dtype=np.float32),
            "bq": np.ascontiguousarray(inputs["bq"], dtype=np.float32).reshape(D, 1),
            "bk": np.ascontiguousarray(inputs["bk"], dtype=np.float32).reshape(D, 1),
            "bv": np.ascontiguousarray(inputs["bv"], dtype=np.float32).reshape(1, C),
        })

    res = run_bass_kernel_spmd(nc, in_maps, list(range(B)))
    LAST_RESULTS = res
    out = np.stack([res.results[b]["out"].reshape(C, HW, HW) for b in range(B)])
    return out.astype(np.float32)



# revision 4
# speedup vs baseline: 1.0374x; 1.0374x over previous
"""Trainium2 Bass kernel for CustomAttention (non-local block).

Reference math (per batch b):
    xf = x.reshape(C, N)                      # C=512, N=H*W=4096
    qT = Wq @ xf + bq                         # [64, N]   (q transposed: d on partitions)
    kT = Wk @ xf + bk                         # [64, N]
    sT[j, i] = sum_d kT[d, j] * qT[d, i]      # scores, keys on partitions
    attn = softmax_j                          # exp(s-3) / Z (shift-invariant)
    vT[n, e] = sum_c xf[c, n] Wv[e, c] + bv   # v transposed: n on partitions
    out[e, i] = gamma * (sum_j vT[j, e] exp_sT[j, i]) / Z[i] + x[e, i]

Sharding: pure data-parallel — batch b -> NeuronCore b (B == 8 == n_cores).

The PV (attn @ V) contraction and the softmax denominator run as
fp8e5 DoubleRow matmuls: K=256 keys per pass (2 fp8 weights per PE
cell), halving the dominant tensor-engine stream cost. exp tiles are
written by the scalar engine directly in fp8e5 with a -3 bias folded
into the activation; V is quantized to fp8e5 once at projection time.
q/k projections and the score matmuls stay float32r (exact).
"""

import numpy as np

import concourse.mybir as mybir
import concourse.tile as tile
from concourse import bacc
from concourse.bass_utils import run_bass_kernel_spmd
from concourse.masks import make_identity

B, C, HW, N, D = 8, 512, 64, 4096, 64
P = 128          # partitions
CB = C // P      # 4 channel chunks
JB = N // P      # 32 key chunks
JP = JB // 2     # 16 key chunk-pairs (DoubleRow: 256 keys per pass)
IB = N // 512    # 8 query blocks
NB = 512         # query block width
F32 = mybir.dt.float32
F32R = mybir.dt.float32r
F8E5 = mybir.dt.float8e5
EXP_SHIFT = -3.0  # exp(s-3): keeps e5m2 in range (max score ~11.9, cap e^10.96)

# exposed for test harness
LAST_RESULTS = None


def build_nc(gamma: float):
    nc = bacc.Bacc(None, target_bir_lowering=False)

    x = nc.dram_tensor("x", [C, N], F32, kind="ExternalInput")
    wq = nc.dram_tensor("Wq", [D, C], F32, kind="ExternalInput")
    wk = nc.dram_tensor("Wk", [D, C], F32, kind="ExternalInput")
    wv = nc.dram_tensor("Wv", [C, C], F32, kind="ExternalInput")
    bq = nc.dram_tensor("bq", [D, 1], F32, kind="ExternalInput")
    bk = nc.dram_tensor("bk", [D, 1], F32, kind="ExternalInput")
    bv = nc.dram_tensor("bv", [1, C], F32, kind="ExternalInput")
    out = nc.dram_tensor("out", [C, N], F32, kind="ExternalOutput")

    # x rows grouped as (c p): chunk c holds rows c*128 .. c*128+127
    x_pcn = x[:, :].rearrange("(c p) n -> p c n", p=P)

    from contextlib import ExitStack

    with tile.TileContext(nc) as tc, ExitStack() as stack:
        const = stack.enter_context(tc.tile_pool(name="const", bufs=1))
        qk_pool = stack.enter_context(tc.tile_pool(name="qk", bufs=1))
        vt_pool = stack.enter_context(tc.tile_pool(name="vt", bufs=1))

        wqkT = const.tile([P, CB, P], F32R, tag="wqkT")  # cols 0-63 Wq^T, 64-127 Wk^T
        wvT = const.tile([P, CB, C], F32R, tag="wvT")
        bqk_sb = const.tile([P, 1], F32, tag="bqk")  # rows 0-63 bq, 64-127 bk
        bvb = const.tile([P, C], F32, tag="bvb")
        ones_dr = const.tile([P, 2, 32], F8E5, tag="ones8")  # DR stationary for Z
        gamma_col = const.tile([1, P], F32R, tag="gam")

        qT = qk_pool.tile([P, N], F32R, tag="qT")
        kT = qk_pool.tile([P, N], F32R, tag="kT")
        # v in fp8e5, key chunk jt on dim 1; DR pairs are adjacent chunks
        vT = vt_pool.tile([P, JB, C], F8E5, tag="vT")

        ones_f32 = const.tile([P, 2, 32], F32, tag="ones_f32")
        nc.vector.memset(ones_f32, 1.0)
        with nc.allow_low_precision(reason="constant ones for fp8 Z matmul"):
            nc.vector.tensor_copy(ones_dr, ones_f32)
        gam_f32 = const.tile([1, P], F32, tag="gam_f32")
        nc.vector.memset(gam_f32, gamma)
        nc.vector.tensor_copy(gamma_col, gam_f32)
        expb = const.tile([P, 1], F32, tag="expb")
        nc.vector.memset(expb, EXP_SHIFT)

        # ---------------- phase 0: weights + q/k/v projections ----------------
        with (
            tc.tile_pool(name="ph0", bufs=1) as ph0,
            tc.tile_pool(name="ph0x", bufs=3) as ph0x,
            tc.tile_pool(name="ph0ps", bufs=1, space="PSUM") as ph0ps,
        ):
            ident = ph0.tile([P, P], F32, tag="ident")
            make_identity(nc, ident)

            wq_raw = ph0.tile([D, C], F32, tag="wq_raw")
            wk_raw = ph0.tile([D, C], F32, tag="wk_raw")
            wv_raw = ph0.tile([P, CB, C], F32, tag="wv_raw")  # [e-part, e-chunk, c]
            nc.gpsimd.dma_start(out=wv_raw, in_=wv[:, :].rearrange("(e p) c -> p e c", p=P))
            nc.sync.dma_start(out=wq_raw, in_=wq[:, :])
            nc.sync.dma_start(out=wk_raw, in_=wk[:, :])
            nc.gpsimd.dma_start(out=bqk_sb[0:D, :], in_=bq[:, :])
            nc.gpsimd.dma_start(out=bqk_sb[D:2 * D, :], in_=bk[:, :])
            nc.gpsimd.dma_start(out=bvb, in_=bv[:, :].to_broadcast((P, C)))

            # transpose Wq/Wk: [64, 128c] -> [128c, 64]; q lands in cols 0-63,
            # k in cols 64-127 of the packed weight
            for c in range(CB):
                pq = ph0ps.tile([P, D], F32, tag="wt", bufs=2)
                nc.tensor.transpose(pq, wq_raw[:, c * P:(c + 1) * P], ident[0:D, 0:D])
                nc.vector.tensor_copy(wqkT[:, c, 0:D], pq)
                pk = ph0ps.tile([P, D], F32, tag="wt", bufs=2)
                nc.tensor.transpose(pk, wk_raw[:, c * P:(c + 1) * P], ident[0:D, 0:D])
                nc.vector.tensor_copy(wqkT[:, c, D:2 * D], pk)
            # transpose Wv blocks: in [e-part, c-cols] -> out [c-part, e-cols]
            for c in range(CB):
                for e in range(CB):
                    pv = ph0ps.tile([P, P], F32, tag="wt", bufs=2)
                    nc.tensor.transpose(pv, wv_raw[:, e, c * P:(c + 1) * P], ident)
                    nc.vector.tensor_copy(wvT[:, c, e * P:(e + 1) * P], pv)

            # projections, one 512-wide n-block at a time
            for nb in range(IB):
                ns = slice(nb * NB, (nb + 1) * NB)
                xt = ph0x.tile([P, CB, NB], F32, tag="xt")
                nc.sync.dma_start(out=xt, in_=x_pcn[:, :, ns])
                xtr = ph0x.tile([P, CB, NB], F32R, tag="xtr")
                nc.vector.tensor_copy(xtr, xt)

                psqk = ph0ps.tile([P, NB], F32, tag="q")
                for c in range(CB):
                    nc.tensor.matmul(psqk, wqkT[:, c, :], xtr[:, c, :],
                                     start=(c == 0), stop=(c == CB - 1))
                # engines are lane-locked: q rows live at psum 0-63, k rows at
                # 64-127, so k is written to the HIGH half of kT
                nc.scalar.activation(qT[0:D, ns], psqk[0:D, :],
                                     mybir.ActivationFunctionType.Identity,
                                     bias=bqk_sb[0:D, :])
                nc.scalar.activation(kT[D:2 * D, ns], psqk[D:2 * D, :],
                                     mybir.ActivationFunctionType.Identity,
                                     bias=bqk_sb[D:2 * D, :])
                # incremental row-duplication so copies overlap the matmuls
                nc.sync.dma_start(out=qT[D:2 * D, ns], in_=qT[0:D, ns])
                nc.sync.dma_start(out=kT[0:D, ns], in_=kT[D:2 * D, ns])
                for sub in range(4):
                    jt = nb * 4 + sub
                    sl = slice(sub * P, (sub + 1) * P)
                    psv = ph0ps.tile([P, C], F32, tag="v", bufs=4)
                    for c in range(CB):
                        nc.tensor.matmul(psv, xtr[:, c, sl], wvT[:, c, :],
                                         start=(c == 0), stop=(c == CB - 1))
                    with nc.allow_low_precision(reason="V quantized to fp8e5 for DoubleRow PV"):
                        nc.vector.tensor_tensor(vT[:, jt, :], psv, bvb,
                                                op=mybir.AluOpType.add)

        # ---------------- main loop: attention ----------------
        with (
            tc.tile_pool(name="expp", bufs=6) as expp,
            tc.tile_pool(name="xst", bufs=6) as xst,
            tc.tile_pool(name="ost", bufs=6) as ost,
            tc.tile_pool(name="small", bufs=2) as small,
            tc.tile_pool(name="mps", bufs=1, space="PSUM") as mps,
        ):
            for ib in range(IB):
                isl = slice(ib * NB, (ib + 1) * NB)
                psum_z = mps.tile([32, NB], F32, tag="z")
                psum_pv = [mps.tile([P, NB], F32, tag=f"pv{e}", name=f"psum_pv{e}")
                           for e in range(CB)]
                exp_tiles = {}

                def consume_z(t, psum_z=psum_z, exp_tiles=exp_tiles):
                    # DoubleRow: one pass sums 256 keys; M=32 stationary of
                    # ones (M=1 drains at half rate on the psum port)
                    nc.tensor.matmul(psum_z, ones_dr, exp_tiles[t],
                                     start=(t == 0), stop=(t == JP - 1),
                                     perf_mode=mybir.MatmulPerfMode.DoubleRow)

                def consume_pv(t, psum_pv=psum_pv, exp_tiles=exp_tiles):
                    et = exp_tiles.pop(t)
                    for e in range(CB):
                        nc.tensor.matmul(psum_pv[e],
                                         vT[:, 2 * t:2 * t + 2, e * P:(e + 1) * P],
                                         et,
                                         start=(t == 0), stop=(t == JP - 1),
                                         perf_mode=mybir.MatmulPerfMode.DoubleRow)

                for t in range(JP):
                    # row-packed pair of K=64 score matmuls (array rows 0-63 /
                    # 64-127) on separate psum banks
                    ja, jb = 2 * t, 2 * t + 1
                    psa = mps.tile([P, NB], F32, tag="s", bufs=3, name="psa")
                    psb = mps.tile([P, NB], F32, tag="s", bufs=3, name="psb")
                    nc.tensor.matmul(psa, kT[0:D, ja * P:(ja + 1) * P],
                                     qT[0:D, isl], start=True, stop=True)
                    nc.tensor.matmul(psb, kT[D:2 * D, jb * P:(jb + 1) * P],
                                     qT[D:2 * D, isl], start=True, stop=True,
                                     tile_position=(D, 0))
                    # exp in fp8e5, DR pair layout [128, 2, NB]: plane 0 = keys
                    # 256t..256t+127, plane 1 = keys 256t+128..256t+255
                    et = expp.tile([P, 2, NB], F8E5, tag="exp", name="et")
                    exp_tiles[t] = et
                    nc.scalar.activation(et[:, 0, :], psa,
                                         mybir.ActivationFunctionType.Exp,
                                         bias=expb)
                    nc.scalar.activation(et[:, 1, :], psb,
                                         mybir.ActivationFunctionType.Exp,
                                         bias=expb)
                    if t >= 1:
                        consume_z(t - 1)
                    if t >= 2:
                        consume_pv(t - 2)
                consume_z(JP - 1)
                consume_pv(JP - 2)
                consume_pv(JP - 1)

                # reciprocal of Z, fold gamma, broadcast across partitions
                # via a K=1 matmul
                rz = small.tile([1, NB], F32R, tag="rz")
                with nc.allow_low_precision(reason="f32r is 32-bit; feeds f32r bcast matmul"):
                    nc.vector.reciprocal(rz, psum_z[0:1, :])
                psrz = mps.tile([P, NB], F32, tag="z")
                nc.tensor.matmul(psrz, gamma_col, rz, start=True, stop=True)
                rzb = small.tile([P, NB], F32, tag="rzb")
                nc.vector.tensor_copy(rzb, psrz)

                for e in range(CB):
                    esl = slice(e * P, (e + 1) * P)
                    xt = xst.tile([P, NB], F32, tag="x")
                    nc.gpsimd.dma_start(out=xt, in_=x[esl, isl])
                    ot = ost.tile([P, NB], F32, tag="o")
                    nc.vector.tensor_tensor(ot, psum_pv[e], rzb,
                                            op=mybir.AluOpType.mult)
                    nc.vector.tensor_tensor(ot, ot, xt, op=mybir.AluOpType.add)
                    nc.sync.dma_start(out=out[esl, isl], in_=ot)

    nc.compile()
    return nc


def kernel(**inputs):
    global LAST_RESULTS
    x = np.asarray(inputs["x"], dtype=np.float32)
    gamma = float(np.asarray(inputs["gamma"]).reshape(-1)[0])

    nc = build_nc(gamma)

    in_maps = []
    for b in range(B):
        in_maps.append({
            "x": np.ascontiguousarray(x[b].reshape(C, N)),
            "Wq": np.ascontiguousarray(inputs["Wq"], dtype=np.float32),
            "Wk": np.ascontiguousarray(inputs["Wk"], dtype=np.float32),
            "Wv": np.ascontiguousarray(inputs["Wv"], dtype=np.float32),
            "bq": np.ascontiguousarray(inputs["bq"], dtype=np.float32).reshape(D, 1),
            "bk": np.ascontiguousarray(inputs["bk"], dtype=np.float32).reshape(D, 1),
            "bv": np.ascontiguousarray(inputs["bv"], dtype=np.float32).reshape(1, C),
        })

    res = run_bass_kernel_spmd(nc, in_maps, list(range(B)))
    LAST_RESULTS = res
    out = np.stack([res.results[b]["out"].reshape(C, HW, HW) for b in range(B)])
    return out.astype(np.float32)
